# revision 32
# baseline (speedup 1.0000x reference)
"""Trainium2 Bass kernel for nn_Encoder_83992380441041 (causal linear attention
encoder, last-position readout).

Math (per segment b of T tokens):
    yn   = LayerNorm(x_b) * gamma + beta          (beta == 0 here)
    K    = phi(yn @ Wk.T); V = yn @ Wv.T; q = phi(yn[T-1] @ Wq.T)
    out  = q @ (K.T V) / (q . sum_t K_t + eps)    [only last position matters]
with phi(a) = elu(a)+1 = min(exp(a),1) + relu(a).

Sharding: data-parallel over segments. 64 segments -> 8 cores x 8 segments.
Weights/LN gamma folded host-side; replicated to all cores.

Per-core schedule (n_tiles tiles of 128 tokens; blocks of 4 tiles; halves of 8):
  x: one big DMA per block
  stats: sum/sumsq via batched tensor_reduce; mu/var/rsqrt chained per half
  xn = (x-mu)*r  (one fused tensor_scalar per tile, DVE/gpsimd alternating)
  xnT via PE transpose, 4 per PSUM bank, one [128,512] copy out per block
  G = xn @ [Wk~|Wv~].T, 4 per PSUM bank
  K = min(exp(Gk),1)+relu(Gk) (ACT only runs Exp+Sqrt; relu on DVE; join on gpsimd)
  S_b|Z_b = K_b.T [V_b|1]; q from xnT last-token columns; nd = [S|Z].T q
  z = num/(den+eps)
"""

import numpy as np

import concourse.bass as bass
import concourse.tile as tile
from concourse import mybir
from concourse.bass_utils import run_bass_kernel_spmd
from concourse.vector_clock import ScopedClock
import bass_rust

EPS_LN = 1e-5
EPS_DEN = 1e-5

F32 = mybir.dt.float32
AF = mybir.ActivationFunctionType
ALU = mybir.AluOpType

N_CORES = 8


def _patched_drain_and_barrier(self, tick_clock, wait_clock):
    # Stock TileContext exit puts one sem-wait per outstanding proc on a
    # single InstDrain; walrus in this container caps sync waits per
    # instruction. Split them across a chain of drains on the same engine
    # (program order preserved => equivalent).
    nc = self.nc
    drain_inst = nc.sync.drain()
    wait_clock.add_sem_waits(
        drain_inst.ins, ScopedClock({None: tick_clock.global_clock})
    )
    si = drain_inst.ins.sync_info
    if si is not None and si.on_wait is not None and len(si.on_wait) > 1:
        waits = list(si.on_wait)
        si.on_wait = waits[:1]
        for w in waits[1:]:
            d2 = nc.sync.drain()
            si2 = d2.ins.sync_info
            if si2 is None:
                d2.ins.sync_info = bass_rust.SyncInfo(on_wait=[w], on_update=[])
            else:
                si2.on_wait = [w]
    nc.all_engine_barrier()
    assert self.sems is not None
    popped = nc._tile_sem_poison_stack.pop()
    assert popped is self._sem_poison
    nc.clear_and_free_semaphores(list(self.sems.allocated().values()))
    nc.all_engine_barrier()


tile.TileContext._drain_and_barrier = _patched_drain_and_barrier

_orig_commit = tile.TileContext._commit_instruction
_wsplit_counter = [0]


def _patched_commit_instruction(self, inst, lazy_reg_writes: bool = True):
    # Enforce the per-instruction sync-wait capacity of the walrus in this
    # container (1 for regular instructions, 2 for EventSemaphore) by
    # spilling excess waits onto same-engine NOPs committed just before.
    si = getattr(inst, "sync_info", None)
    if si is not None and si.on_wait:
        cap = 2 if isinstance(inst, mybir.InstEventSemaphore) else 1
        if len(si.on_wait) > cap:
            waits = list(si.on_wait)
            si.on_wait = waits[:cap]
            for w in waits[cap:]:
                _wsplit_counter[0] += 1
                nop = mybir.InstNoOp(
                    name=f"wsplit-{_wsplit_counter[0]}",
                    sync_info=mybir.SyncInfo(on_wait=[w], on_update=[]),
                    bass_nofuse=True,
                    engine=inst.engine,
                )
                _orig_commit(self, nop, lazy_reg_writes=False)
    return _orig_commit(self, inst, lazy_reg_writes=lazy_reg_writes)


tile.TileContext._commit_instruction = _patched_commit_instruction


def _build(n_tok: int, n_seg: int, d: int, f: int):
    """Per-core program. Inputs: x [n_tok,d]; wkv [d,2f]=[Wk~|Wv~].T;
    wq [d,f]=(Wq~).T; ident [128,128]. Output: z [n_seg,f]."""
    P = 128
    assert n_tok % P == 0 and d == P
    n_tiles = n_tok // P
    t_seg = n_tok // n_seg
    assert t_seg % P == 0
    tiles_per_seg = t_seg // P
    f2 = 2 * f
    B = 4                       # tiles per block (DMA / PSUM-bank batch)
    n_blk = n_tiles // B
    assert n_tiles % B == 0 and n_blk % 2 == 0

    nc = bass.Bass()
    x_d = nc.declare_dram_parameter("x", [n_tok, d], F32, isOutput=False)
    # packed [wkv | wq | ident] -> one DMA
    wpack_d = nc.declare_dram_parameter(
        "wpack", [P, f2 + f + P], F32, isOutput=False
    )
    z_d = nc.declare_dram_parameter("z", [n_seg, f], F32, isOutput=True)

    with tile.TileContext(nc) as tc:
        with (
            tc.tile_pool(name="singles", bufs=1) as singles,
            tc.tile_pool(name="xc", bufs=6) as xcp,
            tc.tile_pool(name="phi", bufs=3) as phip,
            tc.tile_pool(name="sseg", bufs=3) as ssegp,
            tc.tile_pool(name="fin", bufs=1) as finp,
            tc.tile_pool(name="psT", bufs=2, space="PSUM") as psT,
            tc.tile_pool(name="psG", bufs=2, space="PSUM") as psG,
            tc.tile_pool(name="psS", bufs=2, space="PSUM") as psS,
            tc.tile_pool(name="psM", bufs=1, space="PSUM") as psM,
        ):
            # --- persistent buffers ---
            xbig = singles.tile([P, n_tok], F32)
            wpack = singles.tile([P, f2 + f + P], F32)
            xct_big = singles.tile([P, n_tok], F32)
            kbig = singles.tile([P, n_tiles * f], F32)
            vbig = singles.tile([P, n_tiles * (f + 1)], F32)
            bnbig = singles.tile([P, n_tiles, 6], F32)
            mv_big = singles.tile([P, 2 * n_tiles], F32)
            nmr_big = singles.tile([P, n_tiles], F32)
            rbig = singles.tile([P, n_tiles], F32)
            eps_s = singles.tile([P, 1], F32)

            # --- DMA triggers: x first (sync+scalar alternating), then weights
            xsrc = x_d.rearrange("(n p) d -> p n d", p=P)
            for b in range(n_blk):
                eng = nc.sync if b % 2 == 0 else nc.scalar
                eng.dma_start(
                    out=xbig[:, b * B * P:(b + 1) * B * P],
                    in_=xsrc[:, b * B:(b + 1) * B, :],
                )
            nc.scalar.dma_start(out=wpack[:], in_=wpack_d[:])
            wkv_s = wpack[:, 0:f2]
            wq_s = wpack[:, f2:f2 + f]
            ident_s = wpack[:, f2 + f:f2 + f + P]

            nc.vector.memset(eps_s[:], EPS_LN)
            nc.vector.memset(vbig[:, f::(f + 1)], 1.0)

            q8 = finp.tile([f, n_seg], F32)
            q8t = finp.tile([n_seg, f], F32)
            ndsb = finp.tile([f + 1, n_seg], F32)
            znum = finp.tile([n_seg, f + 1], F32)
            zden = finp.tile([n_seg, 1], F32)
            zout = finp.tile([n_seg, f], F32)
            eq = finp.tile([n_seg, f], F32)
            sq8 = finp.tile([n_seg, f], F32)

            xview = xbig[:].rearrange("p (n d) -> p n d", d=P)

            # --- per-block fused chain: stats -> xn -> transpose -> G -> phi -> S ---
            s_sbs = []
            for b in range(n_blk):
                # stats for the block's 4 tiles
                for j in range(B):
                    n = b * B + j
                    nc.vector.bn_stats(out=bnbig[:, n, :], in_=xview[:, n, :])
                    nc.vector.bn_aggr(
                        out=mv_big[:, 2 * n:2 * n + 2], in_=bnbig[:, n, :]
                    )
                b0 = b * B
                rsl = slice(b0, b0 + B)
                nc.scalar.activation(
                    out=rbig[:, rsl],
                    in_=mv_big[:, 2 * b0 + 1:2 * (b0 + B):2],
                    func=AF.Sqrt, bias=eps_s[:], scale=1.0,
                )
                nc.vector.reciprocal(out=rbig[:, rsl], in_=rbig[:, rsl])
                # nmr = -mu * r  (bias for the ACT-side xn computation)
                nc.vector.scalar_tensor_tensor(
                    out=nmr_big[:, rsl],
                    in0=mv_big[:, 2 * b0:2 * (b0 + B):2], scalar=-1.0,
                    in1=rbig[:, rsl], op0=ALU.mult, op1=ALU.mult,
                )

                # xn + transpose, 4 tiles into one PSUM bank
                pT = psT.tile([P, B * P], F32)
                for j in range(B):
                    n = b * B + j
                    xc_t = xcp.tile([P, d], F32)
                    if n % 2 == 0:
                        nc.vector.tensor_scalar(
                            out=xc_t[:], in0=xview[:, n, :],
                            scalar1=mv_big[:, 2 * n:2 * n + 1],
                            scalar2=rbig[:, n:n + 1],
                            op0=ALU.subtract, op1=ALU.mult,
                        )
                    else:
                        # xn = x*r + (-mu*r) on the scalar engine
                        nc.scalar.activation(
                            out=xc_t[:], in_=xview[:, n, :], func=AF.Identity,
                            bias=nmr_big[:, n:n + 1], scale=rbig[:, n:n + 1],
                        )
                    nc.tensor.matmul(
                        pT[:, j * P:(j + 1) * P], lhsT=xc_t[:],
                        rhs=ident_s, is_transpose=True,
                        start=True, stop=True, skip_group_check=True,
                    )
                dst = xct_big[:, b * B * P:(b + 1) * B * P]
                if b % 2 == 0:
                    nc.vector.tensor_copy(out=dst, in_=pT[:])
                else:
                    nc.scalar.copy(out=dst, in_=pT[:])

                # G then phi
                gT = psG.tile([P, B * f2], F32)
                for j in range(B):
                    n = b * B + j
                    nc.tensor.matmul(
                        gT[:, j * f2:(j + 1) * f2],
                        lhsT=xct_big[:, n * P:(n + 1) * P], rhs=wkv_s,
                        start=True, stop=True, skip_group_check=True,
                    )
                gv = gT[:].rearrange("p (j e) -> p j e", e=f2)
                gk_ap = gv[:, :, 0:f]
                gv_ap = gv[:, :, f:f2]
                e_t = phip.tile([P, B * f], F32, tag="e")
                nc.scalar.activation(out=e_t[:], in_=gk_ap, func=AF.Exp)
                s_t = phip.tile([P, B * f], F32, tag="s")
                nc.vector.tensor_scalar_max(out=s_t[:], in0=gk_ap, scalar1=0.0)
                nc.vector.scalar_tensor_tensor(
                    out=kbig[:, b * B * f:(b + 1) * B * f],
                    in0=e_t[:], scalar=1.0, in1=s_t[:],
                    op0=ALU.min, op1=ALU.add,
                )
                vdst = vbig[:, b * B * (f + 1):(b + 1) * B * (f + 1)]
                vdst = vdst.rearrange("p (j c) -> p j c", c=f + 1)[:, :, 0:f]
                if b % 2 == 0:
                    nc.vector.tensor_copy(out=vdst, in_=gv_ap)
                else:
                    nc.scalar.copy(out=vdst, in_=gv_ap)

                # S|Z for the block's two segments (tiles 4b..4b+3)
                assert B == 2 * tiles_per_seg
                s_ps = psS.tile([f, 2 * (f + 1)], F32)
                for hh in range(2):
                    s = 2 * b + hh
                    for j in range(tiles_per_seg):
                        n = s * tiles_per_seg + j
                        nc.tensor.matmul(
                            s_ps[:, hh * (f + 1):(hh + 1) * (f + 1)],
                            lhsT=kbig[:, n * f:(n + 1) * f],
                            rhs=vbig[:, n * (f + 1):(n + 1) * (f + 1)],
                            start=(j == 0), stop=(j == tiles_per_seg - 1),
                            skip_group_check=True,
                        )
                s_sb = ssegp.tile([f, 2 * (f + 1)], F32)
                if b % 2 == 0:
                    nc.vector.tensor_copy(out=s_sb[:], in_=s_ps[:])
                else:
                    nc.scalar.copy(out=s_sb[:], in_=s_ps[:])
                s_sbs.append(s_sb)

            # --- q batch: last-token xn^T columns ---
            xq = xct_big[:, t_seg - 1::t_seg]
            qpT = psM.tile([n_seg, f], F32, tag="m")
            nc.tensor.matmul(qpT[:], lhsT=xq, rhs=wq_s[:], start=True, stop=True)
            nc.scalar.activation(out=eq[:], in_=qpT[:], func=AF.Exp)
            nc.vector.tensor_scalar_max(out=sq8[:], in0=qpT[:], scalar1=0.0)
            nc.vector.scalar_tensor_tensor(
                out=q8t[:], in0=eq[:], scalar=1.0, in1=sq8[:],
                op0=ALU.min, op1=ALU.add,
            )
            q8ps = psM.tile([f, n_seg], F32, tag="m")
            nc.tensor.transpose(q8ps[:], q8t[:], ident_s[0:n_seg, 0:n_seg])
            nc.vector.tensor_copy(out=q8[:], in_=q8ps[:])

            # --- per-segment: S|Z then nd = [S|Z].T q ---
            ndT = psM.tile([f + 1, n_seg], F32, tag="m")
            for u in range(n_seg // 2):
                s_ps = psS.tile([f, 2 * (f + 1)], F32)
                for hh in range(2):
                    s = 2 * u + hh
                    for j in range(tiles_per_seg):
                        n = s * tiles_per_seg + j
                        nc.tensor.matmul(
                            s_ps[:, hh * (f + 1):(hh + 1) * (f + 1)],
                            lhsT=kbig[:, n * f:(n + 1) * f],
                            rhs=vbig[:, n * (f + 1):(n + 1) * (f + 1)],
                            start=(j == 0), stop=(j == tiles_per_seg - 1),
                            skip_group_check=True,
                        )
                s_sb = ssegp.tile([f, 2 * (f + 1)], F32)
                if u % 2 == 0:
                    nc.vector.tensor_copy(out=s_sb[:], in_=s_ps[:])
                else:
                    nc.scalar.copy(out=s_sb[:], in_=s_ps[:])
                for hh in range(2):
                    s = 2 * u + hh
                    nc.tensor.matmul(
                        ndT[:, s:s + 1],
                        lhsT=s_sb[:, hh * (f + 1):(hh + 1) * (f + 1)],
                        rhs=q8[:, s:s + 1],
                        start=True, stop=True, skip_group_check=True,
                    )

            nc.vector.tensor_copy(out=ndsb[:], in_=ndT[:])
            nd_ps = psM.tile([n_seg, f + 1], F32, tag="m")
            nc.tensor.transpose(nd_ps[:], ndsb[:], ident_s[0:f + 1, 0:f + 1])
            nc.vector.tensor_copy(out=znum[:], in_=nd_ps[:])
            nc.vector.tensor_scalar_add(
                out=zden[:], in0=znum[:, f:f + 1], scalar1=EPS_DEN
            )
            nc.vector.reciprocal(out=zden[:], in_=zden[:])
            nc.vector.tensor_scalar_mul(
                out=zout[:], in0=znum[:, :f], scalar1=zden[:]
            )
            nc.sync.dma_start(out=z_d[:], in_=zout[:])

    return nc


def _prep(inputs):
    x = np.ascontiguousarray(np.asarray(inputs["x"], dtype=np.float32))
    batch = np.asarray(inputs["batch"]).astype(np.int64)
    gamma = np.asarray(inputs["gamma"], dtype=np.float32)
    beta = np.asarray(inputs["beta"], dtype=np.float32)
    wk = np.asarray(inputs["Wk"], dtype=np.float32)
    wq = np.asarray(inputs["Wq"], dtype=np.float32)
    wv = np.asarray(inputs["Wv"], dtype=np.float32)
    n_batches = int(np.asarray(inputs["n_batches"]))

    n, d = x.shape
    f = wk.shape[0]
    t_seg = n // n_batches
    counts = np.bincount(batch, minlength=n_batches)
    if not (np.all(counts == t_seg) and np.all(np.diff(batch) >= 0)):
        raise NotImplementedError("kernel specialized for equal sorted segments")
    if np.any(beta != 0.0):
        raise NotImplementedError("kernel specialized for beta == 0")

    wkg = wk * gamma[None, :]
    wvg = wv * gamma[None, :]
    wqg = wq * gamma[None, :]
    wkv_t = np.concatenate([wkg, wvg], axis=0).T            # [d, 2f]
    wq_t = wqg.T                                            # [d, f]
    ident = np.eye(128, dtype=np.float32)
    wpack = np.ascontiguousarray(
        np.concatenate([wkv_t, wq_t, ident], axis=1), dtype=np.float32
    )

    return x, wpack, n, d, f, n_batches, t_seg


def _run(inputs, trace=False):
    x, wpack, n, d, f, n_batches, t_seg = _prep(inputs)

    segs_per_core = n_batches // N_CORES
    tok_per_core = segs_per_core * t_seg
    nc = _build(tok_per_core, segs_per_core, d, f)

    in_maps = []
    for c in range(N_CORES):
        m = {
            "x": np.ascontiguousarray(x[c * tok_per_core:(c + 1) * tok_per_core]),
            "wpack": wpack,
        }
        in_maps.append(m)

    res = run_bass_kernel_spmd(nc, in_maps, list(range(N_CORES)), trace=trace)
    z = np.concatenate([res.results[c]["z"] for c in range(N_CORES)], axis=0)
    return z, res


def kernel(**inputs) -> np.ndarray:
    z, _ = _run(inputs, trace=False)
    return z
